# revision 13
# baseline (speedup 1.0000x reference)
import os
import sys

import numpy as np

for _p in ("/opt/trn_rl_repo", "/root/.axon_site/_ro/trn_rl_repo"):
    if os.path.isdir(_p) and _p not in sys.path:
        sys.path.insert(0, _p)

B, N, D, KD = 32, 2048, 512, 256
F = D + KD
FP = F + 4
NCORES = 8
BL = B // NCORES
NT = 16
NEGM = -70.0

_BUILD_CACHE = {}
last_results = None


def _build():
    import concourse.bass as bass
    import concourse.tile as tile
    from concourse import bacc, mybir
    from concourse.masks import make_identity

    f32 = mybir.dt.float32
    f16 = mybir.dt.float16

    nc = bacc.Bacc()

    x_f = nc.dram_tensor("x_f", [BL, N, FP], f16, kind="ExternalInput")
    x_b = nc.dram_tensor("x_b", [BL, N, FP], f16, kind="ExternalInput")
    SET_COLS = 4 * BL * NT + 2 * FP
    setup_d = nc.dram_tensor("setup", [128, SET_COLS], f16, kind="ExternalInput")
    G0_f = nc.dram_tensor("G0_f", [128, 7, D], f16, kind="ExternalInput")
    G1_f = nc.dram_tensor("G1_f", [128, 7, D], f16, kind="ExternalInput")
    G0_b = nc.dram_tensor("G0_b", [128, 7, D], f16, kind="ExternalInput")
    G1_b = nc.dram_tensor("G1_b", [128, 7, D], f16, kind="ExternalInput")
    out_f = nc.dram_tensor("out_f", [BL, D], f32, kind="ExternalOutput")
    out_b = nc.dram_tensor("out_b", [BL, D], f32, kind="ExternalOutput")

    branches = [
        dict(x=x_f, G0=G0_f, G1=G1_f, out=out_f),
        dict(x=x_b, G0=G0_b, G1=G1_b, out=out_b),
    ]

    with tile.TileContext(nc) as tc:
        with (
            tc.tile_pool(name="singles", bufs=1) as singles,
            tc.tile_pool(name="xp", bufs=4) as xp,
            tc.tile_pool(name="prodp", bufs=3) as prodp,
            tc.tile_pool(name="scr", bufs=3) as scr,
            tc.tile_pool(name="small", bufs=4) as small,
            tc.tile_pool(name="ppp", bufs=3) as ppp,
            tc.tile_pool(name="uallp", bufs=2) as uallp,
            tc.tile_pool(name="uallTp", bufs=2) as uallTp,
            tc.tile_pool(name="finp", bufs=2) as finp,
            tc.tile_pool(name="psU_K", bufs=2, space="PSUM") as psU_K,
            tc.tile_pool(name="psU_1", bufs=2, space="PSUM") as psU_1,
            tc.tile_pool(name="psTr", bufs=2, space="PSUM") as psTr,
            tc.tile_pool(name="psOut", bufs=1, space="PSUM") as psOut,
        ):
            ident = singles.tile([128, 128], f32)
            make_identity(nc, ident)
            ones11 = singles.tile([1, 1], f32)
            nc.vector.memset(ones11, 1.0)

            MW = BL * NT
            setup_t = singles.tile([128, SET_COLS], f16)
            nc.gpsimd.dma_start(out=setup_t, in_=setup_d[:, :])
            per_br = []
            for bi in range(2):
                st = {}
                st["m0"] = setup_t[:, 2 * bi * MW : (2 * bi + 1) * MW]
                st["m1"] = setup_t[:, (2 * bi + 1) * MW : (2 * bi + 2) * MW]
                st["vb"] = setup_t[:, 4 * MW + bi * FP : 4 * MW + (bi + 1) * FP]
                per_br.append(st)

            def load_g(bi):
                br, st = branches[bi], per_br[bi]
                for gname in ("G0", "G1"):
                    g = br[gname]
                    gs = singles.tile([128, 7, D], f16, tag=f"{gname}_{bi}")
                    nc.gpsimd.dma_start(out=gs, in_=g[:, :, :])
                    st[gname] = gs

            NU = 2 * BL
            state = {}

            def stage_a(u):
                bi, b = divmod(u, BL)
                br, st = branches[bi], per_br[bi]
                k = 7 if u % 2 == 0 else 8
                m = NT - k
                xt = xp.tile([128, NT, FP], f16, tag="xt")
                xsrc = br["x"][b].rearrange("(p n) d -> p n d", n=NT)
                if u == 0:
                    nc.gpsimd.dma_start(out=xt[:, 0:m, :], in_=xsrc[:, 0:m, :])
                    nc.gpsimd.dma_start(out=xt[:, m:NT, :], in_=xsrc[:, m:NT, :])
                else:
                    nc.gpsimd.dma_start(out=xt, in_=xsrc)
                pp = ppp.tile([128, NT, 8], f16, tag="pp")
                nc.vector.memset(pp, 0.0)
                sB = small.tile([128, NT], f32, tag="sB")
                prodm = prodp.tile([128, 9, FP], f16, tag="prodm")
                vbb = bass.AP(
                    tensor=st["vb"].tensor,
                    offset=st["vb"].offset,
                    ap=[st["vb"].ap[0]] + [[0, m]] + st["vb"].ap[1:],
                )
                nc.vector.tensor_mul(prodm[:, 0:m, :], xt[:, 0:m, :], vbb)
                for j in range(m):
                    nc.scalar.activation(
                        out=prodm[:, j, :],
                        in_=prodm[:, j, :],
                        func=mybir.ActivationFunctionType.Copy,
                        accum_out=sB[:, j : j + 1],
                    )
                prod = scr.tile([128, FP], f16, tag="prod")
                for n in range(m, NT):
                    nc.vector.scalar_tensor_tensor(
                        out=prod,
                        in0=xt[:, n, :],
                        scalar=0.0,
                        in1=st["vb"],
                        op0=mybir.AluOpType.bypass,
                        op1=mybir.AluOpType.mult,
                        accum_out=sB[:, n : n + 1],
                    )
                state[u] = (xt, pp, sB)

            def stage_b(u):
                bi, b = divmod(u, BL)
                br, st = branches[bi], per_br[bi]
                xt, pp, sB = state.pop(u)
                s0 = small.tile([128, NT], f16, tag="s0")
                s1 = small.tile([128, NT], f16, tag="s1")
                nc.vector.tensor_add(s0, sB, st["m0"][:, b * NT : (b + 1) * NT])
                nc.vector.tensor_add(s1, sB, st["m1"][:, b * NT : (b + 1) * NT])
                nc.scalar.activation(
                    out=pp[:, :, b], in_=s0, func=mybir.ActivationFunctionType.Exp
                )
                nc.scalar.activation(
                    out=pp[:, :, 4 + b], in_=s1, func=mybir.ActivationFunctionType.Exp
                )
                psK, ps1 = state["ps", bi]
                for n in range(NT):
                    first = b == 0 and n == 0
                    last = b == BL - 1 and n == NT - 1
                    nc.tensor.matmul(
                        psK, pp[:, n, :], xt[:, n, 0:D], start=first, stop=last
                    )
                    nc.tensor.matmul(
                        ps1, pp[:, n, :], xt[:, n, D:FP], start=first, stop=last
                    )

            def finishing(bi):
                br, st = branches[bi], per_br[bi]
                psK, ps1 = state.pop(("ps", bi))
                uall = uallp.tile([8, F + 1], f32)
                nc.scalar.activation(out=uall[:, 0:D], in_=psK,
                                     func=mybir.ActivationFunctionType.Copy)
                nc.scalar.activation(out=uall[:, D : F + 1], in_=ps1[:, 0 : KD + 1],
                                     func=mybir.ActivationFunctionType.Copy)

                uallT = uallTp.tile([128, 7, 8], f16)
                for k in range(6):
                    trp = psTr.tile([128, 8], f32)
                    nc.tensor.transpose(trp, uall[:, k * 128 : (k + 1) * 128], ident[0:8, 0:8])
                    nc.scalar.activation(out=uallT[:, k, :], in_=trp,
                                         func=mybir.ActivationFunctionType.Copy)
                trp = psTr.tile([128, 8], f32)
                nc.tensor.transpose(trp[0:1, :], uall[:, F : F + 1], ident[0:8, 0:8])
                nc.scalar.activation(out=uallT[0:1, 6, :], in_=trp[0:1, :],
                                      func=mybir.ActivationFunctionType.Copy)
                pT = finp.tile([1, 8], f32, tag="pT")
                nc.scalar.activation(out=pT, in_=trp[0:1, :],
                                     func=mybir.ActivationFunctionType.Copy)

                po = psOut.tile([4, D + 1], f32)
                for k in range(6):
                    nc.tensor.matmul(
                        po[:, 0:D], uallT[:, k, 0:4], st["G0"][:, k, :], start=(k == 0), stop=False
                    )
                nc.tensor.matmul(
                    po[:, 0:D], uallT[0:1, 6, 0:4], st["G0"][0:1, 6, :], start=False, stop=False
                )
                for k in range(6):
                    nc.tensor.matmul(
                        po[:, 0:D], uallT[:, k, 4:8], st["G1"][:, k, :], start=False, stop=False
                    )
                nc.tensor.matmul(
                    po[:, 0:D], uallT[0:1, 6, 4:8], st["G1"][0:1, 6, :], start=False, stop=True
                )
                nc.tensor.matmul(po[:, D : D + 1], pT[:, 0:4], ones11, start=True, stop=False)
                nc.tensor.matmul(po[:, D : D + 1], pT[:, 4:8], ones11, start=False, stop=True)

                rp = finp.tile([4, 1], f32, tag="rp")
                nc.vector.reciprocal(rp, po[:, D : D + 1])
                osb = finp.tile([4, D], f32, tag="osb")
                nc.vector.tensor_scalar_mul(out=osb, in0=po[:, 0:D], scalar1=rp)
                nc.sync.dma_start(out=br["out"][:, :], in_=osb)

            for bi in range(2):
                psK = psU_K.tile([8, D], f32, tag="psK")
                ps1 = psU_1.tile([8, KD + 4], f32, tag="ps1")
                state["ps", bi] = (psK, ps1)
            for u in range(NU + 1):
                if u >= 1:
                    stage_b(u - 1)
                if u < NU:
                    stage_a(u)
                if u == 3:
                    load_g(0)
                if u == BL + 1:
                    load_g(1)
                if u >= 1 and (u - 1) % BL == BL - 1:
                    finishing((u - 1) // BL)

    nc.compile()
    return nc


def _get_nc():
    if "nc" not in _BUILD_CACHE:
        _BUILD_CACHE["nc"] = _build()
    return _BUILD_CACHE["nc"]


def _pack_x(Kv, k1):
    x = np.empty((B, N, FP), np.float16)
    x[:, :, 0:D] = Kv
    x[:, :, D:F] = k1
    x[:, :, F : F + 2] = 1.0
    x[:, :, F + 2 : FP] = 0.0
    return x


def kernel(**inputs) -> tuple:
    global last_results
    from concourse.bass_utils import run_bass_kernel_spmd

    f32 = np.float32
    f16 = np.float16
    Wfk = np.asarray(inputs["Wfk"], dtype=f32)
    bfk = np.asarray(inputs["bfk"], dtype=f32)
    Wbk = np.asarray(inputs["Wbk"], dtype=f32)
    bbk = np.asarray(inputs["bbk"], dtype=f32)
    Wr0 = np.asarray(inputs["Wr0"], dtype=f32)
    Wr1 = np.asarray(inputs["Wr1"], dtype=f32)
    wf_den = np.asarray(inputs["wf_den"], dtype=f32)
    wb_den = np.asarray(inputs["wb_den"], dtype=f32)
    i = int(np.asarray(inputs["i"]))
    num_utter = int(np.asarray(inputs["num_utter"]))

    x_f = _pack_x(np.asarray(inputs["K"]), np.asarray(inputs["front_k1"]))
    x_b = _pack_x(np.asarray(inputs["back_K"]), np.asarray(inputs["back_k2"]))

    adj_f = np.asarray(inputs["front_sdj_den"], dtype=f32)
    sm_f = np.asarray(inputs["front_s_mask"], dtype=f32)
    adj_b = np.asarray(inputs["back_sdj_den"], dtype=f32)
    sm_b = np.asarray(inputs["back_s_mask"], dtype=f32)

    def pack_mask(m):
        m = (NEGM * (1.0 - m)).astype(f16)
        m = m.reshape(NCORES, BL, 128, NT)
        return np.ascontiguousarray(m.transpose(0, 2, 1, 3))

    m0_f = pack_mask(adj_f * sm_f)
    m1_f = pack_mask(adj_f * (1.0 - sm_f))
    m0_b = pack_mask(adj_b * sm_b)
    m1_b = pack_mask(adj_b * (1.0 - sm_b))

    def fold_v(Wk, wden):
        v = np.zeros((FP,), f16)
        v[0:F] = (Wk.astype(np.float64) @ wden[D:].astype(np.float64)).astype(f16)
        return v

    v_f = fold_v(Wfk, wf_den)
    v_b = fold_v(Wbk, wb_den)
    A_f = np.vstack([Wfk, bfk[None, :]]).astype(np.float64)
    A_b = np.vstack([Wbk, bbk[None, :]]).astype(np.float64)
    def pack_g(G):
        gs = np.zeros((128, 7, D), f16)
        gs[:, 0:6, :] = G[0:F].reshape(6, 128, D).transpose(1, 0, 2)
        gs[0, 6, :] = G[F]
        return gs

    G0_f = pack_g((A_f @ Wr0.astype(np.float64)).astype(f16))
    G1_f = pack_g((A_f @ Wr1.astype(np.float64)).astype(f16))
    G0_b = pack_g((A_b @ Wr0.astype(np.float64)).astype(f16))
    G1_b = pack_g((A_b @ Wr1.astype(np.float64)).astype(f16))

    nc = _get_nc()

    MW = BL * NT
    SET_COLS = 4 * MW + 2 * FP
    in_maps = []
    for c in range(NCORES):
        s = slice(c * BL, (c + 1) * BL)
        setup = np.empty((128, SET_COLS), f16)
        setup[:, 0 * MW : 1 * MW] = m0_f[c].reshape(128, MW)
        setup[:, 1 * MW : 2 * MW] = m1_f[c].reshape(128, MW)
        setup[:, 2 * MW : 3 * MW] = m0_b[c].reshape(128, MW)
        setup[:, 3 * MW : 4 * MW] = m1_b[c].reshape(128, MW)
        setup[:, 4 * MW : 4 * MW + FP] = v_f
        setup[:, 4 * MW + FP :] = v_b
        in_maps.append(
            {
                "x_f": x_f[s],
                "x_b": x_b[s],
                "setup": setup,
                "G0_f": G0_f,
                "G1_f": G1_f,
                "G0_b": G0_b,
                "G1_b": G1_b,
            }
        )

    trace = os.environ.get("KERNEL_TRACE", "0") == "1"
    res = run_bass_kernel_spmd(nc, in_maps, core_ids=list(range(NCORES)), trace=trace)
    last_results = res

    front = np.concatenate([r["out_f"] for r in res.results], axis=0)
    back = np.concatenate([r["out_b"] for r in res.results], axis=0)
    if i == 0:
        front = np.zeros((B, D), dtype=f32)
    if i == num_utter - 1:
        back = np.zeros((B, D), dtype=f32)
    return (front, back)


# revision 15
# speedup vs baseline: 1.1062x; 1.1062x over previous
import os
import sys

import numpy as np

for _p in ("/opt/trn_rl_repo", "/root/.axon_site/_ro/trn_rl_repo"):
    if os.path.isdir(_p) and _p not in sys.path:
        sys.path.insert(0, _p)

B, N, D, KD = 32, 2048, 512, 256
F = D + KD
FP = F + 4
NCORES = 8
BL = B // NCORES
NT = 16
NEGM = -70.0

_BUILD_CACHE = {}
last_results = None


def _build():
    import concourse.bass as bass
    import concourse.tile as tile
    from concourse import bacc, mybir
    from concourse.masks import make_identity

    f32 = mybir.dt.float32
    f16 = mybir.dt.float16

    nc = bacc.Bacc()

    x_f = nc.dram_tensor("x_f", [BL, N, FP], f16, kind="ExternalInput")
    x_b = nc.dram_tensor("x_b", [BL, N, FP], f16, kind="ExternalInput")
    SET_COLS = 4 * BL * NT + 2 * FP
    setup_d = nc.dram_tensor("setup", [128, SET_COLS], f16, kind="ExternalInput")
    G0_f = nc.dram_tensor("G0_f", [128, 7, D], f16, kind="ExternalInput")
    G1_f = nc.dram_tensor("G1_f", [128, 7, D], f16, kind="ExternalInput")
    G0_b = nc.dram_tensor("G0_b", [128, 7, D], f16, kind="ExternalInput")
    G1_b = nc.dram_tensor("G1_b", [128, 7, D], f16, kind="ExternalInput")
    out_f = nc.dram_tensor("out_f", [BL, D], f32, kind="ExternalOutput")
    out_b = nc.dram_tensor("out_b", [BL, D], f32, kind="ExternalOutput")

    branches = [
        dict(x=x_f, G0=G0_f, G1=G1_f, out=out_f),
        dict(x=x_b, G0=G0_b, G1=G1_b, out=out_b),
    ]

    with tile.TileContext(nc) as tc:
        with (
            tc.tile_pool(name="singles", bufs=1) as singles,
            tc.tile_pool(name="xp", bufs=3) as xp,
            tc.tile_pool(name="x0p", bufs=1) as x0p,
            tc.tile_pool(name="prodp", bufs=3) as prodp,
            tc.tile_pool(name="scr", bufs=3) as scr,
            tc.tile_pool(name="small", bufs=4) as small,
            tc.tile_pool(name="ppp", bufs=3) as ppp,
            tc.tile_pool(name="uallp", bufs=2) as uallp,
            tc.tile_pool(name="uallTp", bufs=2) as uallTp,
            tc.tile_pool(name="finp", bufs=2) as finp,
            tc.tile_pool(name="psU_K", bufs=2, space="PSUM") as psU_K,
            tc.tile_pool(name="psU_1", bufs=2, space="PSUM") as psU_1,
            tc.tile_pool(name="psTr", bufs=2, space="PSUM") as psTr,
            tc.tile_pool(name="psOut", bufs=1, space="PSUM") as psOut,
        ):
            ident = singles.tile([128, 128], f32)
            make_identity(nc, ident)
            ones11 = singles.tile([1, 1], f32)
            nc.vector.memset(ones11, 1.0)

            MW = BL * NT
            setup_t = singles.tile([128, SET_COLS], f16)
            nc.gpsimd.dma_start(out=setup_t, in_=setup_d[:, :])
            per_br = []
            for bi in range(2):
                st = {}
                st["m0"] = setup_t[:, 2 * bi * MW : (2 * bi + 1) * MW]
                st["m1"] = setup_t[:, (2 * bi + 1) * MW : (2 * bi + 2) * MW]
                st["vb"] = setup_t[:, 4 * MW + bi * FP : 4 * MW + (bi + 1) * FP]
                per_br.append(st)

            def load_g(bi):
                br, st = branches[bi], per_br[bi]
                for gname in ("G0", "G1"):
                    g = br[gname]
                    gs = singles.tile([128, 7, D], f16, tag=f"{gname}_{bi}")
                    nc.sync.dma_start(out=gs, in_=g[:, :, :])
                    st[gname] = gs

            NU = 2 * BL
            state = {}

            def stage_a(u):
                bi, b = divmod(u, BL)
                br, st = branches[bi], per_br[bi]
                k = 7 if u % 2 == 0 else 8
                m = NT - k
                xsrc = br["x"][b].rearrange("(p n) d -> p n d", n=NT)
                if u == 0:
                    xta = x0p.tile([128, 9, FP], f16, tag="xta")
                    xtb = x0p.tile([128, NT - 9, FP], f16, tag="xtb")
                    nc.gpsimd.dma_start(out=xta, in_=xsrc[:, 0:m, :])
                    nc.gpsimd.dma_start(out=xtb, in_=xsrc[:, m:NT, :])
                    xtile = lambda n: (xta[:, n, :] if n < m else xtb[:, n - m, :])
                else:
                    xt = xp.tile([128, NT, FP], f16, tag="xt")
                    nc.gpsimd.dma_start(out=xt, in_=xsrc)
                    xtile = lambda n: xt[:, n, :]
                pp = ppp.tile([128, NT, 8], f16, tag="pp")
                nc.vector.memset(pp, 0.0)
                sB = small.tile([128, NT], f32, tag="sB")
                prodm = prodp.tile([128, 9, FP], f16, tag="prodm")
                vbb = bass.AP(
                    tensor=st["vb"].tensor,
                    offset=st["vb"].offset,
                    ap=[st["vb"].ap[0]] + [[0, m]] + st["vb"].ap[1:],
                )
                if u == 0:
                    nc.vector.tensor_mul(prodm[:, 0:m, :], xta[:, :, :], vbb)
                else:
                    nc.vector.tensor_mul(prodm[:, 0:m, :], xt[:, 0:m, :], vbb)
                for j in range(m):
                    nc.scalar.activation(
                        out=prodm[:, j, :],
                        in_=prodm[:, j, :],
                        func=mybir.ActivationFunctionType.Copy,
                        accum_out=sB[:, j : j + 1],
                    )
                prod = scr.tile([128, FP], f16, tag="prod")
                for n in range(m, NT):
                    nc.vector.scalar_tensor_tensor(
                        out=prod,
                        in0=xtile(n),
                        scalar=0.0,
                        in1=st["vb"],
                        op0=mybir.AluOpType.bypass,
                        op1=mybir.AluOpType.mult,
                        accum_out=sB[:, n : n + 1],
                    )
                state[u] = (xtile, pp, sB)

            def stage_b(u):
                bi, b = divmod(u, BL)
                br, st = branches[bi], per_br[bi]
                xtile, pp, sB = state.pop(u)
                s0 = small.tile([128, NT], f16, tag="s0")
                s1 = small.tile([128, NT], f16, tag="s1")
                nc.vector.tensor_add(s0, sB, st["m0"][:, b * NT : (b + 1) * NT])
                nc.vector.tensor_add(s1, sB, st["m1"][:, b * NT : (b + 1) * NT])
                nc.scalar.activation(
                    out=pp[:, :, b], in_=s0, func=mybir.ActivationFunctionType.Exp
                )
                nc.scalar.activation(
                    out=pp[:, :, 4 + b], in_=s1, func=mybir.ActivationFunctionType.Exp
                )
                psK, ps1 = state["ps", bi]
                for n in range(NT):
                    first = b == 0 and n == 0
                    last = b == BL - 1 and n == NT - 1
                    xn = xtile(n)
                    nc.tensor.matmul(
                        psK, pp[:, n, :], xn[:, 0:D], start=first, stop=last
                    )
                    nc.tensor.matmul(
                        ps1, pp[:, n, :], xn[:, D:FP], start=first, stop=last
                    )

            def finishing(bi):
                br, st = branches[bi], per_br[bi]
                psK, ps1 = state.pop(("ps", bi))
                uall = uallp.tile([8, F + 1], f32)
                nc.scalar.activation(out=uall[:, 0:D], in_=psK,
                                     func=mybir.ActivationFunctionType.Copy)
                nc.scalar.activation(out=uall[:, D : F + 1], in_=ps1[:, 0 : KD + 1],
                                     func=mybir.ActivationFunctionType.Copy)

                uallT = uallTp.tile([128, 7, 8], f16)
                for k in range(6):
                    trp = psTr.tile([128, 8], f32)
                    nc.tensor.transpose(trp, uall[:, k * 128 : (k + 1) * 128], ident[0:8, 0:8])
                    nc.scalar.activation(out=uallT[:, k, :], in_=trp,
                                         func=mybir.ActivationFunctionType.Copy)
                trp = psTr.tile([128, 8], f32)
                nc.tensor.transpose(trp[0:1, :], uall[:, F : F + 1], ident[0:8, 0:8])
                nc.scalar.activation(out=uallT[0:1, 6, :], in_=trp[0:1, :],
                                      func=mybir.ActivationFunctionType.Copy)
                pT = finp.tile([1, 8], f32, tag="pT")
                nc.scalar.activation(out=pT, in_=trp[0:1, :],
                                     func=mybir.ActivationFunctionType.Copy)

                po = psOut.tile([4, D + 1], f32)
                for k in range(6):
                    nc.tensor.matmul(
                        po[:, 0:D], uallT[:, k, 0:4], st["G0"][:, k, :], start=(k == 0), stop=False
                    )
                nc.tensor.matmul(
                    po[:, 0:D], uallT[0:1, 6, 0:4], st["G0"][0:1, 6, :], start=False, stop=False
                )
                for k in range(6):
                    nc.tensor.matmul(
                        po[:, 0:D], uallT[:, k, 4:8], st["G1"][:, k, :], start=False, stop=False
                    )
                nc.tensor.matmul(
                    po[:, 0:D], uallT[0:1, 6, 4:8], st["G1"][0:1, 6, :], start=False, stop=True
                )
                nc.tensor.matmul(po[:, D : D + 1], pT[:, 0:4], ones11, start=True, stop=False)
                nc.tensor.matmul(po[:, D : D + 1], pT[:, 4:8], ones11, start=False, stop=True)

                rp = finp.tile([4, 1], f32, tag="rp")
                nc.vector.reciprocal(rp, po[:, D : D + 1])
                osb = finp.tile([4, D], f32, tag="osb")
                nc.vector.tensor_scalar_mul(out=osb, in0=po[:, 0:D], scalar1=rp)
                nc.sync.dma_start(out=br["out"][:, :], in_=osb)

            for bi in range(2):
                psK = psU_K.tile([8, D], f32, tag="psK")
                ps1 = psU_1.tile([8, KD + 4], f32, tag="ps1")
                state["ps", bi] = (psK, ps1)
            for u in range(NU + 1):
                if u >= 1:
                    stage_b(u - 1)
                if u < NU:
                    stage_a(u)
                if u == 3:
                    load_g(0)
                if u == BL + 1:
                    load_g(1)
                if u == NU - 1:
                    finishing(0)
                if u == NU:
                    finishing(1)

    nc.compile()
    return nc


def _get_nc():
    if "nc" not in _BUILD_CACHE:
        _BUILD_CACHE["nc"] = _build()
    return _BUILD_CACHE["nc"]


def _pack_x(Kv, k1):
    x = np.empty((B, N, FP), np.float16)
    x[:, :, 0:D] = Kv
    x[:, :, D:F] = k1
    x[:, :, F : F + 2] = 1.0
    x[:, :, F + 2 : FP] = 0.0
    return x


def kernel(**inputs) -> tuple:
    global last_results
    from concourse.bass_utils import run_bass_kernel_spmd

    f32 = np.float32
    f16 = np.float16
    Wfk = np.asarray(inputs["Wfk"], dtype=f32)
    bfk = np.asarray(inputs["bfk"], dtype=f32)
    Wbk = np.asarray(inputs["Wbk"], dtype=f32)
    bbk = np.asarray(inputs["bbk"], dtype=f32)
    Wr0 = np.asarray(inputs["Wr0"], dtype=f32)
    Wr1 = np.asarray(inputs["Wr1"], dtype=f32)
    wf_den = np.asarray(inputs["wf_den"], dtype=f32)
    wb_den = np.asarray(inputs["wb_den"], dtype=f32)
    i = int(np.asarray(inputs["i"]))
    num_utter = int(np.asarray(inputs["num_utter"]))

    x_f = _pack_x(np.asarray(inputs["K"]), np.asarray(inputs["front_k1"]))
    x_b = _pack_x(np.asarray(inputs["back_K"]), np.asarray(inputs["back_k2"]))

    adj_f = np.asarray(inputs["front_sdj_den"], dtype=f32)
    sm_f = np.asarray(inputs["front_s_mask"], dtype=f32)
    adj_b = np.asarray(inputs["back_sdj_den"], dtype=f32)
    sm_b = np.asarray(inputs["back_s_mask"], dtype=f32)

    def pack_mask(m):
        m = (NEGM * (1.0 - m)).astype(f16)
        m = m.reshape(NCORES, BL, 128, NT)
        return np.ascontiguousarray(m.transpose(0, 2, 1, 3))

    m0_f = pack_mask(adj_f * sm_f)
    m1_f = pack_mask(adj_f * (1.0 - sm_f))
    m0_b = pack_mask(adj_b * sm_b)
    m1_b = pack_mask(adj_b * (1.0 - sm_b))

    def fold_v(Wk, wden):
        v = np.zeros((FP,), f16)
        v[0:F] = (Wk.astype(np.float64) @ wden[D:].astype(np.float64)).astype(f16)
        return v

    v_f = fold_v(Wfk, wf_den)
    v_b = fold_v(Wbk, wb_den)
    A_f = np.vstack([Wfk, bfk[None, :]]).astype(np.float64)
    A_b = np.vstack([Wbk, bbk[None, :]]).astype(np.float64)
    def pack_g(G):
        gs = np.zeros((128, 7, D), f16)
        gs[:, 0:6, :] = G[0:F].reshape(6, 128, D).transpose(1, 0, 2)
        gs[0, 6, :] = G[F]
        return gs

    G0_f = pack_g((A_f @ Wr0.astype(np.float64)).astype(f16))
    G1_f = pack_g((A_f @ Wr1.astype(np.float64)).astype(f16))
    G0_b = pack_g((A_b @ Wr0.astype(np.float64)).astype(f16))
    G1_b = pack_g((A_b @ Wr1.astype(np.float64)).astype(f16))

    nc = _get_nc()

    MW = BL * NT
    SET_COLS = 4 * MW + 2 * FP
    in_maps = []
    for c in range(NCORES):
        s = slice(c * BL, (c + 1) * BL)
        setup = np.empty((128, SET_COLS), f16)
        setup[:, 0 * MW : 1 * MW] = m0_f[c].reshape(128, MW)
        setup[:, 1 * MW : 2 * MW] = m1_f[c].reshape(128, MW)
        setup[:, 2 * MW : 3 * MW] = m0_b[c].reshape(128, MW)
        setup[:, 3 * MW : 4 * MW] = m1_b[c].reshape(128, MW)
        setup[:, 4 * MW : 4 * MW + FP] = v_f
        setup[:, 4 * MW + FP :] = v_b
        in_maps.append(
            {
                "x_f": x_f[s],
                "x_b": x_b[s],
                "setup": setup,
                "G0_f": G0_f,
                "G1_f": G1_f,
                "G0_b": G0_b,
                "G1_b": G1_b,
            }
        )

    trace = os.environ.get("KERNEL_TRACE", "0") == "1"
    res = run_bass_kernel_spmd(nc, in_maps, core_ids=list(range(NCORES)), trace=trace)
    last_results = res

    front = np.concatenate([r["out_f"] for r in res.results], axis=0)
    back = np.concatenate([r["out_b"] for r in res.results], axis=0)
    if i == 0:
        front = np.zeros((B, D), dtype=f32)
    if i == num_utter - 1:
        back = np.zeros((B, D), dtype=f32)
    return (front, back)


# revision 17
# speedup vs baseline: 1.1594x; 1.0481x over previous
import os
import sys

import numpy as np

for _p in ("/opt/trn_rl_repo", "/root/.axon_site/_ro/trn_rl_repo"):
    if os.path.isdir(_p) and _p not in sys.path:
        sys.path.insert(0, _p)

B, N, D, KD = 32, 2048, 512, 256
F = D + KD
FP = F + 4
NCORES = 8
BL = B // NCORES
NT = 16
NEGM = -70.0

_BUILD_CACHE = {}
last_results = None


def _build():
    import concourse.bass as bass
    import concourse.tile as tile
    from concourse import bacc, mybir
    from concourse.masks import make_identity

    f32 = mybir.dt.float32
    f16 = mybir.dt.float16

    nc = bacc.Bacc()

    x_f = nc.dram_tensor("x_f", [BL, N, FP], f16, kind="ExternalInput")
    x_b = nc.dram_tensor("x_b", [BL, N, FP], f16, kind="ExternalInput")
    SET_COLS = 4 * BL * NT + 2 * FP
    setup_d = nc.dram_tensor("setup", [128, SET_COLS], f16, kind="ExternalInput")
    G0_f = nc.dram_tensor("G0_f", [128, 7, D], f16, kind="ExternalInput")
    G1_f = nc.dram_tensor("G1_f", [128, 7, D], f16, kind="ExternalInput")
    G0_b = nc.dram_tensor("G0_b", [128, 7, D], f16, kind="ExternalInput")
    G1_b = nc.dram_tensor("G1_b", [128, 7, D], f16, kind="ExternalInput")
    out_f = nc.dram_tensor("out_f", [BL, D], f32, kind="ExternalOutput")
    out_b = nc.dram_tensor("out_b", [BL, D], f32, kind="ExternalOutput")

    branches = [
        dict(x=x_f, G0=G0_f, G1=G1_f, out=out_f),
        dict(x=x_b, G0=G0_b, G1=G1_b, out=out_b),
    ]

    with tile.TileContext(nc) as tc:
        with (
            tc.tile_pool(name="singles", bufs=1) as singles,
            tc.tile_pool(name="xp", bufs=3) as xp,
            tc.tile_pool(name="x0p", bufs=1) as x0p,
            tc.tile_pool(name="prodp", bufs=3) as prodp,
            tc.tile_pool(name="scr", bufs=3) as scr,
            tc.tile_pool(name="small", bufs=4) as small,
            tc.tile_pool(name="ppp", bufs=3) as ppp,
            tc.tile_pool(name="uallp", bufs=2) as uallp,
            tc.tile_pool(name="uallTp", bufs=2) as uallTp,
            tc.tile_pool(name="finp", bufs=2) as finp,
            tc.tile_pool(name="psU_K", bufs=2, space="PSUM") as psU_K,
            tc.tile_pool(name="psU_1", bufs=2, space="PSUM") as psU_1,
            tc.tile_pool(name="psTr", bufs=2, space="PSUM") as psTr,
            tc.tile_pool(name="psOut", bufs=1, space="PSUM") as psOut,
        ):
            ident = singles.tile([128, 128], f32)
            make_identity(nc, ident)
            ones11 = singles.tile([1, 1], f32)
            nc.vector.memset(ones11, 1.0)

            MW = BL * NT
            setup_t = singles.tile([128, SET_COLS], f16)
            nc.gpsimd.dma_start(out=setup_t, in_=setup_d[:, :])
            per_br = []
            for bi in range(2):
                st = {}
                st["m0"] = setup_t[:, 2 * bi * MW : (2 * bi + 1) * MW]
                st["m1"] = setup_t[:, (2 * bi + 1) * MW : (2 * bi + 2) * MW]
                st["vb"] = setup_t[:, 4 * MW + bi * FP : 4 * MW + (bi + 1) * FP]
                per_br.append(st)

            def load_g(bi):
                br, st = branches[bi], per_br[bi]
                for gname in ("G0", "G1"):
                    g = br[gname]
                    gs = singles.tile([128, 7, D], f16, tag=f"{gname}_{bi}")
                    nc.gpsimd.dma_start(out=gs, in_=g[:, :, :])
                    st[gname] = gs

            NU = 2 * BL
            state = {}

            def stage_a(u):
                bi, b = divmod(u, BL)
                br, st = branches[bi], per_br[bi]
                k = 7 if u % 2 == 0 else 8
                m = NT - k
                xsrc = br["x"][b].rearrange("(p n) d -> p n d", n=NT)
                if u == 0:
                    xta = x0p.tile([128, 9, FP], f16, tag="xta")
                    xtb = x0p.tile([128, NT - 9, FP], f16, tag="xtb")
                    nc.gpsimd.dma_start(out=xta, in_=xsrc[:, 0:m, :])
                    nc.gpsimd.dma_start(out=xtb, in_=xsrc[:, m:NT, :])
                    xtile = lambda n: (xta[:, n, :] if n < m else xtb[:, n - m, :])
                    xfirst = xta[:, :, :]
                else:
                    xt = xp.tile([128, NT, FP], f16, tag="xt")
                    nc.gpsimd.dma_start(out=xt, in_=xsrc)
                    xtile = lambda n: xt[:, n, :]
                    xfirst = xt[:, 0:m, :]
                pp = ppp.tile([128, NT, 8], f16, tag="pp")
                nc.vector.memset(pp, 0.0)
                sB = small.tile([128, NT], f32, tag="sB")
                prodm = prodp.tile([128, 9, FP], f16, tag="prodm")
                vb = st["vb"]
                vbb = bass.AP(
                    tensor=vb.tensor,
                    offset=vb.offset,
                    ap=[vb.ap[0]] + [[0, m]] + vb.ap[1:],
                )
                nc.vector.tensor_mul(prodm[:, 0:m, :], xfirst, vbb)
                for j in range(m):
                    nc.scalar.activation(
                        out=prodm[:, j, :],
                        in_=prodm[:, j, :],
                        func=mybir.ActivationFunctionType.Copy,
                        accum_out=sB[:, j : j + 1],
                    )
                prod = scr.tile([128, FP], f16, tag="prod")
                for n in range(m, NT):
                    nc.vector.scalar_tensor_tensor(
                        out=prod,
                        in0=xtile(n),
                        scalar=0.0,
                        in1=st["vb"],
                        op0=mybir.AluOpType.bypass,
                        op1=mybir.AluOpType.mult,
                        accum_out=sB[:, n : n + 1],
                    )
                state[u] = (xtile, pp, sB)

            def stage_b(u):
                bi, b = divmod(u, BL)
                br, st = branches[bi], per_br[bi]
                xtile, pp, sB = state.pop(u)
                s0 = small.tile([128, NT], f16, tag="s0")
                s1 = small.tile([128, NT], f16, tag="s1")
                nc.vector.tensor_add(s0, sB, st["m0"][:, b * NT : (b + 1) * NT])
                nc.vector.tensor_add(s1, sB, st["m1"][:, b * NT : (b + 1) * NT])
                nc.scalar.activation(
                    out=pp[:, :, b], in_=s0, func=mybir.ActivationFunctionType.Exp
                )
                nc.scalar.activation(
                    out=pp[:, :, 4 + b], in_=s1, func=mybir.ActivationFunctionType.Exp
                )
                psK, ps1 = state["ps", bi]
                for n in range(NT):
                    first = b == 0 and n == 0
                    last = b == BL - 1 and n == NT - 1
                    xn = xtile(n)
                    nc.tensor.matmul(
                        psK, pp[:, n, :], xn[:, 0:D], start=first, stop=last
                    )
                    nc.tensor.matmul(
                        ps1, pp[:, n, :], xn[:, D:FP], start=first, stop=last
                    )

            def fin_stage1(bi):
                psK, ps1 = state.pop(("ps", bi))
                uall = uallp.tile([8, F + 1], f32, tag="uall")
                nc.scalar.activation(out=uall[:, 0:D], in_=psK,
                                     func=mybir.ActivationFunctionType.Copy)
                nc.scalar.activation(out=uall[:, D : F + 1], in_=ps1[:, 0 : KD + 1],
                                     func=mybir.ActivationFunctionType.Copy)
                state["uall", bi] = uall

            def fin_stage2(bi):
                uall = state.pop(("uall", bi))
                uallT = uallTp.tile([128, 7, 8], f16, tag="uallT")
                for k in range(6):
                    trp = psTr.tile([128, 8], f32, tag="trp")
                    nc.tensor.transpose(trp, uall[:, k * 128 : (k + 1) * 128], ident[0:8, 0:8])
                    nc.scalar.activation(out=uallT[:, k, :], in_=trp,
                                         func=mybir.ActivationFunctionType.Copy)
                trp = psTr.tile([128, 8], f32, tag="trp")
                nc.tensor.transpose(trp[0:1, :], uall[:, F : F + 1], ident[0:8, 0:8])
                nc.scalar.activation(out=uallT[0:1, 6, :], in_=trp[0:1, :],
                                     func=mybir.ActivationFunctionType.Copy)
                pT = finp.tile([1, 8], f32, tag="pT")
                nc.scalar.activation(out=pT, in_=trp[0:1, :],
                                     func=mybir.ActivationFunctionType.Copy)
                state["uT", bi] = (uallT, pT)

            def fin_stage3(bi):
                br, st = branches[bi], per_br[bi]
                uallT, pT = state.pop(("uT", bi))
                po = psOut.tile([4, D + 1], f32, tag="po")
                for k in range(6):
                    nc.tensor.matmul(
                        po[:, 0:D], uallT[:, k, 0:4], st["G0"][:, k, :], start=(k == 0), stop=False
                    )
                nc.tensor.matmul(
                    po[:, 0:D], uallT[0:1, 6, 0:4], st["G0"][0:1, 6, :], start=False, stop=False
                )
                for k in range(6):
                    nc.tensor.matmul(
                        po[:, 0:D], uallT[:, k, 4:8], st["G1"][:, k, :], start=False, stop=False
                    )
                nc.tensor.matmul(
                    po[:, 0:D], uallT[0:1, 6, 4:8], st["G1"][0:1, 6, :], start=False, stop=True
                )
                nc.tensor.matmul(po[:, D : D + 1], pT[:, 0:4], ones11, start=True, stop=False)
                nc.tensor.matmul(po[:, D : D + 1], pT[:, 4:8], ones11, start=False, stop=True)

                rp = finp.tile([4, 1], f32, tag="rp")
                nc.vector.reciprocal(rp, po[:, D : D + 1])
                osb = finp.tile([4, D], f32, tag="osb")
                nc.vector.tensor_scalar_mul(out=osb, in0=po[:, 0:D], scalar1=rp)
                nc.sync.dma_start(out=br["out"][:, :], in_=osb)

            for bi in range(2):
                psK = psU_K.tile([8, D], f32, tag="psK")
                ps1 = psU_1.tile([8, KD + 4], f32, tag="ps1")
                state["ps", bi] = (psK, ps1)

            for u in range(NU + 1):
                if u >= 1:
                    stage_b(u - 1)
                if u < NU:
                    stage_a(u)
                if u == 3:
                    load_g(0)
                if u == NU - 1:
                    load_g(1)
                if u == BL + 1:
                    fin_stage1(0)
                if u == BL + 2:
                    fin_stage2(0)
                if u == BL + 3:
                    fin_stage3(0)
                if u == NU:
                    fin_stage1(1)
                    fin_stage2(1)
                    fin_stage3(1)

    nc.compile()
    return nc


def _get_nc():
    if "nc" not in _BUILD_CACHE:
        _BUILD_CACHE["nc"] = _build()
    return _BUILD_CACHE["nc"]


def _pack_x(Kv, k1):
    x = np.empty((B, N, FP), np.float16)
    x[:, :, 0:D] = Kv
    x[:, :, D:F] = k1
    x[:, :, F : F + 2] = 1.0
    x[:, :, F + 2 : FP] = 0.0
    return x


def kernel(**inputs) -> tuple:
    global last_results
    from concourse.bass_utils import run_bass_kernel_spmd

    f32 = np.float32
    f16 = np.float16
    Wfk = np.asarray(inputs["Wfk"], dtype=f32)
    bfk = np.asarray(inputs["bfk"], dtype=f32)
    Wbk = np.asarray(inputs["Wbk"], dtype=f32)
    bbk = np.asarray(inputs["bbk"], dtype=f32)
    Wr0 = np.asarray(inputs["Wr0"], dtype=f32)
    Wr1 = np.asarray(inputs["Wr1"], dtype=f32)
    wf_den = np.asarray(inputs["wf_den"], dtype=f32)
    wb_den = np.asarray(inputs["wb_den"], dtype=f32)
    i = int(np.asarray(inputs["i"]))
    num_utter = int(np.asarray(inputs["num_utter"]))

    x_f = _pack_x(np.asarray(inputs["K"]), np.asarray(inputs["front_k1"]))
    x_b = _pack_x(np.asarray(inputs["back_K"]), np.asarray(inputs["back_k2"]))

    adj_f = np.asarray(inputs["front_sdj_den"], dtype=f32)
    sm_f = np.asarray(inputs["front_s_mask"], dtype=f32)
    adj_b = np.asarray(inputs["back_sdj_den"], dtype=f32)
    sm_b = np.asarray(inputs["back_s_mask"], dtype=f32)

    def pack_mask(m):
        m = (NEGM * (1.0 - m)).astype(f16)
        m = m.reshape(NCORES, BL, 128, NT)
        return np.ascontiguousarray(m.transpose(0, 2, 1, 3))

    m0_f = pack_mask(adj_f * sm_f)
    m1_f = pack_mask(adj_f * (1.0 - sm_f))
    m0_b = pack_mask(adj_b * sm_b)
    m1_b = pack_mask(adj_b * (1.0 - sm_b))

    def fold_v(Wk, wden):
        v = np.zeros((FP,), f16)
        v[0:F] = (Wk.astype(np.float64) @ wden[D:].astype(np.float64)).astype(f16)
        return v

    v_f = fold_v(Wfk, wf_den)
    v_b = fold_v(Wbk, wb_den)
    A_f = np.vstack([Wfk, bfk[None, :]]).astype(np.float64)
    A_b = np.vstack([Wbk, bbk[None, :]]).astype(np.float64)

    def pack_g(G):
        gs = np.zeros((128, 7, D), f16)
        gs[:, 0:6, :] = G[0:F].reshape(6, 128, D).transpose(1, 0, 2)
        gs[0, 6, :] = G[F]
        return gs

    G0_f = pack_g((A_f @ Wr0.astype(np.float64)).astype(f16))
    G1_f = pack_g((A_f @ Wr1.astype(np.float64)).astype(f16))
    G0_b = pack_g((A_b @ Wr0.astype(np.float64)).astype(f16))
    G1_b = pack_g((A_b @ Wr1.astype(np.float64)).astype(f16))

    nc = _get_nc()

    MW = BL * NT
    SET_COLS = 4 * MW + 2 * FP
    in_maps = []
    for c in range(NCORES):
        s = slice(c * BL, (c + 1) * BL)
        setup = np.empty((128, SET_COLS), f16)
        setup[:, 0 * MW : 1 * MW] = m0_f[c].reshape(128, MW)
        setup[:, 1 * MW : 2 * MW] = m1_f[c].reshape(128, MW)
        setup[:, 2 * MW : 3 * MW] = m0_b[c].reshape(128, MW)
        setup[:, 3 * MW : 4 * MW] = m1_b[c].reshape(128, MW)
        setup[:, 4 * MW : 4 * MW + FP] = v_f
        setup[:, 4 * MW + FP :] = v_b
        in_maps.append(
            {
                "x_f": x_f[s],
                "x_b": x_b[s],
                "setup": setup,
                "G0_f": G0_f,
                "G1_f": G1_f,
                "G0_b": G0_b,
                "G1_b": G1_b,
            }
        )

    trace = os.environ.get("KERNEL_TRACE", "0") == "1"
    res = run_bass_kernel_spmd(nc, in_maps, core_ids=list(range(NCORES)), trace=trace)
    last_results = res

    front = np.concatenate([r["out_f"] for r in res.results], axis=0)
    back = np.concatenate([r["out_b"] for r in res.results], axis=0)
    if i == 0:
        front = np.zeros((B, D), dtype=f32)
    if i == num_utter - 1:
        back = np.zeros((B, D), dtype=f32)
    return (front, back)


# revision 18
# speedup vs baseline: 1.1638x; 1.0038x over previous
import os
import sys

import numpy as np

for _p in ("/opt/trn_rl_repo", "/root/.axon_site/_ro/trn_rl_repo"):
    if os.path.isdir(_p) and _p not in sys.path:
        sys.path.insert(0, _p)

B, N, D, KD = 32, 2048, 512, 256
F = D + KD
FP = F + 4
NCORES = 8
BL = B // NCORES
NT = 16
NEGM = -70.0

_BUILD_CACHE = {}
last_results = None


def _build():
    import concourse.bass as bass
    import concourse.tile as tile
    from concourse import bacc, mybir
    from concourse.masks import make_identity

    f32 = mybir.dt.float32
    f16 = mybir.dt.float16

    nc = bacc.Bacc()

    x_f = nc.dram_tensor("x_f", [BL, N, FP], f16, kind="ExternalInput")
    x_b = nc.dram_tensor("x_b", [BL, N, FP], f16, kind="ExternalInput")
    SET_COLS = 4 * BL * NT + 2 * FP
    setup_d = nc.dram_tensor("setup", [128, SET_COLS], f16, kind="ExternalInput")
    G0_f = nc.dram_tensor("G0_f", [128, 7, D], f16, kind="ExternalInput")
    G1_f = nc.dram_tensor("G1_f", [128, 7, D], f16, kind="ExternalInput")
    G0_b = nc.dram_tensor("G0_b", [128, 7, D], f16, kind="ExternalInput")
    G1_b = nc.dram_tensor("G1_b", [128, 7, D], f16, kind="ExternalInput")
    out_f = nc.dram_tensor("out_f", [BL, D], f32, kind="ExternalOutput")
    out_b = nc.dram_tensor("out_b", [BL, D], f32, kind="ExternalOutput")

    branches = [
        dict(x=x_f, G0=G0_f, G1=G1_f, out=out_f),
        dict(x=x_b, G0=G0_b, G1=G1_b, out=out_b),
    ]

    with tile.TileContext(nc) as tc:
        with (
            tc.tile_pool(name="singles", bufs=1) as singles,
            tc.tile_pool(name="xp", bufs=3) as xp,
            tc.tile_pool(name="x0p", bufs=1) as x0p,
            tc.tile_pool(name="prodp", bufs=3) as prodp,
            tc.tile_pool(name="scr", bufs=3) as scr,
            tc.tile_pool(name="small", bufs=4) as small,
            tc.tile_pool(name="ppp", bufs=3) as ppp,
            tc.tile_pool(name="uallp", bufs=2) as uallp,
            tc.tile_pool(name="uallTp", bufs=2) as uallTp,
            tc.tile_pool(name="finp", bufs=2) as finp,
            tc.tile_pool(name="psU_K", bufs=2, space="PSUM") as psU_K,
            tc.tile_pool(name="psU_1", bufs=2, space="PSUM") as psU_1,
            tc.tile_pool(name="psTr", bufs=2, space="PSUM") as psTr,
            tc.tile_pool(name="psOut", bufs=1, space="PSUM") as psOut,
        ):
            ident = singles.tile([128, 128], f32)
            make_identity(nc, ident)
            ones11 = singles.tile([1, 1], f32)
            nc.vector.memset(ones11, 1.0)

            MW = BL * NT
            setup_t = singles.tile([128, SET_COLS], f16)
            nc.gpsimd.dma_start(out=setup_t, in_=setup_d[:, :])
            per_br = []
            for bi in range(2):
                st = {}
                st["m0"] = setup_t[:, 2 * bi * MW : (2 * bi + 1) * MW]
                st["m1"] = setup_t[:, (2 * bi + 1) * MW : (2 * bi + 2) * MW]
                st["vb"] = setup_t[:, 4 * MW + bi * FP : 4 * MW + (bi + 1) * FP]
                per_br.append(st)

            def load_g(bi):
                br, st = branches[bi], per_br[bi]
                for gname in ("G0", "G1"):
                    g = br[gname]
                    gs = singles.tile([128, 7, D], f16, tag=f"{gname}_{bi}")
                    nc.gpsimd.dma_start(out=gs, in_=g[:, :, :])
                    st[gname] = gs

            items = [
                dict(bi=0, b=0, t0=0, nt=8, m=5),
                dict(bi=0, b=0, t0=8, nt=8, m=4),
            ] + [
                dict(bi=u // BL, b=u % BL, t0=0, nt=NT, m=9)
                for u in range(1, 2 * BL)
            ]
            NI = len(items)
            state = {}

            def stage_a(ix):
                it = items[ix]
                bi, b, t0, nt, m = it["bi"], it["b"], it["t0"], it["nt"], it["m"]
                br, st = branches[bi], per_br[bi]
                xsrc = br["x"][b].rearrange("(p n) d -> p n d", n=NT)
                if ix <= 1:
                    xh = x0p.tile([128, 8, FP], f16, tag=f"xh{ix}")
                    nc.gpsimd.dma_start(out=xh, in_=xsrc[:, t0 : t0 + nt, :])
                    xt = xh
                else:
                    xt = xp.tile([128, NT, FP], f16, tag="xt")
                    nc.gpsimd.dma_start(out=xt, in_=xsrc)
                pp = ppp.tile([128, NT, 8], f16, tag="pp")
                nc.vector.memset(pp[:, 0:nt, :], 0.0)
                sB = small.tile([128, NT], f32, tag="sB")
                prodm = prodp.tile([128, 9, FP], f16, tag="prodm")
                vb = st["vb"]
                vbb = bass.AP(
                    tensor=vb.tensor,
                    offset=vb.offset,
                    ap=[vb.ap[0]] + [[0, m]] + vb.ap[1:],
                )
                nc.vector.tensor_mul(prodm[:, 0:m, :], xt[:, 0:m, :], vbb)
                for j in range(m):
                    nc.scalar.activation(
                        out=prodm[:, j, :],
                        in_=prodm[:, j, :],
                        func=mybir.ActivationFunctionType.Copy,
                        accum_out=sB[:, j : j + 1],
                    )
                prod = scr.tile([128, FP], f16, tag="prod")
                for n in range(m, nt):
                    nc.vector.scalar_tensor_tensor(
                        out=prod,
                        in0=xt[:, n, :],
                        scalar=0.0,
                        in1=st["vb"],
                        op0=mybir.AluOpType.bypass,
                        op1=mybir.AluOpType.mult,
                        accum_out=sB[:, n : n + 1],
                    )
                state[ix] = (xt, pp, sB)

            def stage_b(ix):
                it = items[ix]
                bi, b, t0, nt = it["bi"], it["b"], it["t0"], it["nt"]
                br, st = branches[bi], per_br[bi]
                xt, pp, sB = state.pop(ix)
                first_it = ix == (0 if bi == 0 else BL + 1)
                last_it = ix == (BL if bi == 0 else NI - 1)
                s0 = small.tile([128, NT], f16, tag="s0")
                s1 = small.tile([128, NT], f16, tag="s1")
                mo = b * NT + t0
                nc.vector.tensor_add(s0[:, 0:nt], sB[:, 0:nt], st["m0"][:, mo : mo + nt])
                nc.vector.tensor_add(s1[:, 0:nt], sB[:, 0:nt], st["m1"][:, mo : mo + nt])
                nc.scalar.activation(
                    out=pp[:, 0:nt, b], in_=s0[:, 0:nt], func=mybir.ActivationFunctionType.Exp
                )
                nc.scalar.activation(
                    out=pp[:, 0:nt, 4 + b], in_=s1[:, 0:nt], func=mybir.ActivationFunctionType.Exp
                )
                psK, ps1 = state["ps", bi]
                for n in range(nt):
                    first = first_it and n == 0
                    last = last_it and n == nt - 1
                    nc.tensor.matmul(
                        psK, pp[:, n, :], xt[:, n, 0:D], start=first, stop=last
                    )
                    nc.tensor.matmul(
                        ps1, pp[:, n, :], xt[:, n, D:FP], start=first, stop=last
                    )

            def fin_stage1(bi):
                psK, ps1 = state.pop(("ps", bi))
                uall = uallp.tile([8, F + 1], f32, tag="uall")
                nc.scalar.activation(out=uall[:, 0:D], in_=psK,
                                     func=mybir.ActivationFunctionType.Copy)
                nc.scalar.activation(out=uall[:, D : F + 1], in_=ps1[:, 0 : KD + 1],
                                     func=mybir.ActivationFunctionType.Copy)
                state["uall", bi] = uall

            def fin_stage2(bi):
                uall = state.pop(("uall", bi))
                uallT = uallTp.tile([128, 7, 8], f16, tag="uallT")
                for k in range(6):
                    trp = psTr.tile([128, 8], f32, tag="trp")
                    nc.tensor.transpose(trp, uall[:, k * 128 : (k + 1) * 128], ident[0:8, 0:8])
                    nc.scalar.activation(out=uallT[:, k, :], in_=trp,
                                         func=mybir.ActivationFunctionType.Copy)
                trp = psTr.tile([128, 8], f32, tag="trp")
                nc.tensor.transpose(trp[0:1, :], uall[:, F : F + 1], ident[0:8, 0:8])
                nc.scalar.activation(out=uallT[0:1, 6, :], in_=trp[0:1, :],
                                     func=mybir.ActivationFunctionType.Copy)
                pT = finp.tile([1, 8], f32, tag="pT")
                nc.scalar.activation(out=pT, in_=trp[0:1, :],
                                     func=mybir.ActivationFunctionType.Copy)
                state["uT", bi] = (uallT, pT)

            def fin_stage3(bi):
                br, st = branches[bi], per_br[bi]
                uallT, pT = state.pop(("uT", bi))
                po = psOut.tile([4, D + 1], f32, tag="po")
                for k in range(6):
                    nc.tensor.matmul(
                        po[:, 0:D], uallT[:, k, 0:4], st["G0"][:, k, :], start=(k == 0), stop=False
                    )
                nc.tensor.matmul(
                    po[:, 0:D], uallT[0:1, 6, 0:4], st["G0"][0:1, 6, :], start=False, stop=False
                )
                for k in range(6):
                    nc.tensor.matmul(
                        po[:, 0:D], uallT[:, k, 4:8], st["G1"][:, k, :], start=False, stop=False
                    )
                nc.tensor.matmul(
                    po[:, 0:D], uallT[0:1, 6, 4:8], st["G1"][0:1, 6, :], start=False, stop=True
                )
                nc.tensor.matmul(po[:, D : D + 1], pT[:, 0:4], ones11, start=True, stop=False)
                nc.tensor.matmul(po[:, D : D + 1], pT[:, 4:8], ones11, start=False, stop=True)

                rp = finp.tile([4, 1], f32, tag="rp")
                nc.vector.reciprocal(rp, po[:, D : D + 1])
                osb = finp.tile([4, D], f32, tag="osb")
                nc.vector.tensor_scalar_mul(out=osb, in0=po[:, 0:D], scalar1=rp)
                nc.sync.dma_start(out=br["out"][:, :], in_=osb)

            for bi in range(2):
                psK = psU_K.tile([8, D], f32, tag="psK")
                ps1 = psU_1.tile([8, KD + 4], f32, tag="ps1")
                state["ps", bi] = (psK, ps1)

            for i in range(NI + 1):
                if i >= 1:
                    stage_b(i - 1)
                if i < NI:
                    stage_a(i)
                if i == 4:
                    load_g(0)
                if i == NI - 1:
                    load_g(1)
                if i == BL + 2:
                    fin_stage1(0)
                if i == BL + 3:
                    fin_stage2(0)
                if i == BL + 4:
                    fin_stage3(0)
                if i == NI:
                    fin_stage1(1)
                    fin_stage2(1)
                    fin_stage3(1)

    nc.compile()
    return nc


def _get_nc():
    if "nc" not in _BUILD_CACHE:
        _BUILD_CACHE["nc"] = _build()
    return _BUILD_CACHE["nc"]


def _pack_x(Kv, k1):
    x = np.empty((B, N, FP), np.float16)
    x[:, :, 0:D] = Kv
    x[:, :, D:F] = k1
    x[:, :, F : F + 2] = 1.0
    x[:, :, F + 2 : FP] = 0.0
    return x


def kernel(**inputs) -> tuple:
    global last_results
    from concourse.bass_utils import run_bass_kernel_spmd

    f32 = np.float32
    f16 = np.float16
    Wfk = np.asarray(inputs["Wfk"], dtype=f32)
    bfk = np.asarray(inputs["bfk"], dtype=f32)
    Wbk = np.asarray(inputs["Wbk"], dtype=f32)
    bbk = np.asarray(inputs["bbk"], dtype=f32)
    Wr0 = np.asarray(inputs["Wr0"], dtype=f32)
    Wr1 = np.asarray(inputs["Wr1"], dtype=f32)
    wf_den = np.asarray(inputs["wf_den"], dtype=f32)
    wb_den = np.asarray(inputs["wb_den"], dtype=f32)
    i = int(np.asarray(inputs["i"]))
    num_utter = int(np.asarray(inputs["num_utter"]))

    x_f = _pack_x(np.asarray(inputs["K"]), np.asarray(inputs["front_k1"]))
    x_b = _pack_x(np.asarray(inputs["back_K"]), np.asarray(inputs["back_k2"]))

    adj_f = np.asarray(inputs["front_sdj_den"], dtype=f32)
    sm_f = np.asarray(inputs["front_s_mask"], dtype=f32)
    adj_b = np.asarray(inputs["back_sdj_den"], dtype=f32)
    sm_b = np.asarray(inputs["back_s_mask"], dtype=f32)

    def pack_mask(m):
        m = (NEGM * (1.0 - m)).astype(f16)
        m = m.reshape(NCORES, BL, 128, NT)
        return np.ascontiguousarray(m.transpose(0, 2, 1, 3))

    m0_f = pack_mask(adj_f * sm_f)
    m1_f = pack_mask(adj_f * (1.0 - sm_f))
    m0_b = pack_mask(adj_b * sm_b)
    m1_b = pack_mask(adj_b * (1.0 - sm_b))

    def fold_v(Wk, wden):
        v = np.zeros((FP,), f16)
        v[0:F] = (Wk.astype(np.float64) @ wden[D:].astype(np.float64)).astype(f16)
        return v

    v_f = fold_v(Wfk, wf_den)
    v_b = fold_v(Wbk, wb_den)
    A_f = np.vstack([Wfk, bfk[None, :]]).astype(np.float64)
    A_b = np.vstack([Wbk, bbk[None, :]]).astype(np.float64)

    def pack_g(G):
        gs = np.zeros((128, 7, D), f16)
        gs[:, 0:6, :] = G[0:F].reshape(6, 128, D).transpose(1, 0, 2)
        gs[0, 6, :] = G[F]
        return gs

    G0_f = pack_g((A_f @ Wr0.astype(np.float64)).astype(f16))
    G1_f = pack_g((A_f @ Wr1.astype(np.float64)).astype(f16))
    G0_b = pack_g((A_b @ Wr0.astype(np.float64)).astype(f16))
    G1_b = pack_g((A_b @ Wr1.astype(np.float64)).astype(f16))

    nc = _get_nc()

    MW = BL * NT
    SET_COLS = 4 * MW + 2 * FP
    in_maps = []
    for c in range(NCORES):
        s = slice(c * BL, (c + 1) * BL)
        setup = np.empty((128, SET_COLS), f16)
        setup[:, 0 * MW : 1 * MW] = m0_f[c].reshape(128, MW)
        setup[:, 1 * MW : 2 * MW] = m1_f[c].reshape(128, MW)
        setup[:, 2 * MW : 3 * MW] = m0_b[c].reshape(128, MW)
        setup[:, 3 * MW : 4 * MW] = m1_b[c].reshape(128, MW)
        setup[:, 4 * MW : 4 * MW + FP] = v_f
        setup[:, 4 * MW + FP :] = v_b
        in_maps.append(
            {
                "x_f": x_f[s],
                "x_b": x_b[s],
                "setup": setup,
                "G0_f": G0_f,
                "G1_f": G1_f,
                "G0_b": G0_b,
                "G1_b": G1_b,
            }
        )

    trace = os.environ.get("KERNEL_TRACE", "0") == "1"
    res = run_bass_kernel_spmd(nc, in_maps, core_ids=list(range(NCORES)), trace=trace)
    last_results = res

    front = np.concatenate([r["out_f"] for r in res.results], axis=0)
    back = np.concatenate([r["out_b"] for r in res.results], axis=0)
    if i == 0:
        front = np.zeros((B, D), dtype=f32)
    if i == num_utter - 1:
        back = np.zeros((B, D), dtype=f32)
    return (front, back)
